# revision 19
# baseline (speedup 1.0000x reference)
"""MoE routing kernel for one TRN2 chip (8 NeuronCores).

Strategy: expert parallelism, one expert per core. Host-side dispatch:
tokens are grouped by expert (deduped via per-(token,expert) combine
weights), padded to a common capacity C, and packed into PE-friendly
layouts. Each core runs the full gate/up/silu/mul/down MLP for its
expert in bf16 (f32 PSUM accumulation); the host applies combine
weights and scatter-adds into the final output.

All matmuls keep tokens on the moving (free) dimension:
  gate/up: psum[I-chunk 128, ctok]  = Wg_tile[K=H-chunk, M=I-chunk].T @ xT[K, ctok]
  down:    psum[H-chunk 128, ctok]  = Wd_tile[K=I-chunk, M=H-chunk].T @ hidT[K, ctok]
so no on-chip transposes are needed anywhere.
"""

import numpy as np
import ml_dtypes

import concourse.bacc as bacc
import concourse.mybir as mybir
import concourse.tile as tile
from concourse.bass_utils import run_bass_kernel_spmd


def _prune_redundant_ldweights(ordered_by_block):
    """Drop InstLdweights whose stationary AP is identical to the weights
    already resident in the PE array (loaded by the previous Ldweights in the
    same block, with only Matmults in between). Runs on tile_legalize output,
    before semaphore assignment, so sync stays consistent. The PE array
    content can't change between the kept load and the elided one: any SBUF
    overwrite of the weight buffer is ordered after the consuming Matmults,
    which still carry the weights AP in their ins.
    """
    n_pruned = 0
    for bb, insts in list(ordered_by_block.items()):
        loaded = None
        out = []
        for inst in insts:
            tn = type(inst).__name__
            if tn == "InstLdweights":
                sig = (
                    str(inst.ins[0]),
                    str(inst.tile_position),
                    str(inst.tile_size),
                    str(inst.perf_mode),
                    str(inst.is_transpose),
                )
                if sig == loaded:
                    n_pruned += 1
                    continue
                loaded = sig
            elif tn == "InstMatmult":
                if inst.is_transpose:
                    loaded = None  # transpose clobbers the array
            out.append(inst)
        ordered_by_block[bb] = out
    return n_pruned


class _LegalizeWithPrune:
    def __init__(self, orig):
        self.orig = orig

    def __call__(self, ordered, nc):
        out = self.orig(ordered, nc)
        _prune_redundant_ldweights(out)
        return out

H = 1024
I = 4096
E = 8
HP = H // 128   # 8  H-chunks
IP = I // 128   # 32 I-chunks
NMAX = 512      # matmul moving free-dim chunk (one PSUM bank of f32)

BF16 = mybir.dt.bfloat16
F32 = mybir.dt.float32
_bf16 = ml_dtypes.bfloat16

_PROGRAM_CACHE = {}


def _chunks(C):
    out = []
    c0 = 0
    while c0 < C:
        n = min(NMAX, C - c0)
        out.append((c0, n))
        c0 += n
    return out


def build_program(C, niter=1, prune=True, order="gguu", interleave_m=1,
                  balance=False, dtype=None, head=True, xbufs=2, **kw):
    """One-core program (SPMD across 8 cores). C = token capacity per core."""
    key = (C, niter, prune, order, interleave_m, balance, str(dtype),
           head, xbufs, tuple(sorted(kw.items())))
    if key in _PROGRAM_CACHE:
        return _PROGRAM_CACHE[key]

    _orig_legalize = tile.tile_legalize
    if prune:
        tile.tile_legalize = _LegalizeWithPrune(_orig_legalize)
    try:
        nc = _build_program_inner(C, niter, order, interleave_m, balance,
                                  dtype or BF16, head=head, xbufs=xbufs, **kw)
    finally:
        tile.tile_legalize = _orig_legalize
    _PROGRAM_CACHE[key] = nc
    return nc


def _build_program_inner(C, niter, order="gguu", interleave_m=1,
                         balance=False, BF16=BF16, xbufs=1, wbufs=4,
                         dbufs=3, head=False):
    nc = bacc.Bacc("TRN2", target_bir_lowering=False, debug=False, num_devices=8)
    xp = nc.dram_tensor("xp", [128, HP, C], BF16, kind="ExternalInput").ap()
    wg = nc.dram_tensor("wg", [IP, 128, HP * 128], BF16, kind="ExternalInput").ap()
    wu = nc.dram_tensor("wu", [IP, 128, HP * 128], BF16, kind="ExternalInput").ap()
    wd = nc.dram_tensor("wd", [HP, 128, IP * 128], BF16, kind="ExternalInput").ap()
    out = nc.dram_tensor("out", [HP, 128, C], F32, kind="ExternalOutput").ap()

    if balance:
        half = ((C + 15) // 16) * 8
        chunks = [(0, half), (half, C - half)]
    else:
        chunks = _chunks(C)

    with tile.TileContext(nc) as tc:
        with (
            tc.tile_pool(name="xpool", bufs=xbufs) as xpool,
            tc.tile_pool(name="hpool", bufs=1) as hpool,
            tc.tile_pool(name="wgpool", bufs=wbufs) as wgpool,
            tc.tile_pool(name="wupool", bufs=wbufs) as wupool,
            tc.tile_pool(name="wdpool", bufs=dbufs) as wdpool,
            tc.tile_pool(name="sgpool", bufs=3) as sgpool,
            tc.tile_pool(name="stpool", bufs=3) as stpool,
            tc.tile_pool(name="otpool", bufs=3) as otpool,
            tc.tile_pool(name="pspool", bufs=8, space="PSUM") as pspool,
        ):

            def body(_iv=None):
                # Optionally land the first weight tiles before the bulk x
                # DMA so the PE's first Ldweights isn't queued behind 2MB.
                pre_w = []
                if head:
                    wgt0 = wgpool.tile([128, HP * 128], BF16, name="wgt",
                                       tag="wgt")
                    nc.sync.dma_start(wgt0[:], wg[0])
                    wut0 = wupool.tile([128, HP * 128], BF16, name="wut",
                                       tag="wut")
                    nc.sync.dma_start(wut0[:], wu[0])
                    pre_w = [(wgt0, wut0)]
                xs = xpool.tile([128, HP, C], BF16, name="xs", tag="xs")
                for k in range(HP):
                    nc.sync.dma_start(xs[:, k, :], xp[:, k, :])
                hid = hpool.tile([128, IP, C], BF16, name="hid", tag="hid")

                # ---- phase 1: hidT[i, c] = silu(gateT) * upT ----
                # Token chunks innermost so each loaded weight tile serves
                # all chunks before the PE switches weights.
                for im in range(IP):
                    if im < len(pre_w):
                        wgt, wut = pre_w[im]
                    else:
                        wgt = wgpool.tile([128, HP * 128], BF16, name="wgt",
                                          tag="wgt")
                        nc.sync.dma_start(wgt[:], wg[im])
                        wut = wupool.tile([128, HP * 128], BF16, name="wut",
                                          tag="wut")
                        nc.sync.dma_start(wut[:], wu[im])
                    pgs = [pspool.tile([128, NMAX], F32, name="psg", tag="ps")
                           for _ in chunks]
                    pus = [pspool.tile([128, NMAX], F32, name="psu", tag="ps")
                           for _ in chunks]
                    for k in range(HP):
                        if order == "gguu":
                            seq = [(pgs, wgt, ci) for ci in range(len(chunks))]
                            seq += [(pus, wut, ci) for ci in range(len(chunks))]
                        else:  # "gugu": alternate stationary every matmul
                            seq = []
                            for ci in range(len(chunks)):
                                seq.append((pgs, wgt, ci))
                                seq.append((pus, wut, ci))
                        for pss, wt, ci in seq:
                            c0, n = chunks[ci]
                            nc.tensor.matmul(
                                pss[ci][:, :n],
                                wt[:, k * 128:(k + 1) * 128],
                                xs[:, k, c0:c0 + n],
                                start=(k == 0),
                                stop=(k == HP - 1),
                            )
                    for ci, (c0, n) in enumerate(chunks):
                        sg = sgpool.tile([128, NMAX], F32, name="sg", tag="sg")
                        nc.scalar.activation(
                            sg[:, :n], pgs[ci][:, :n],
                            mybir.ActivationFunctionType.Sigmoid,
                        )
                        st = stpool.tile([128, NMAX], F32, name="st", tag="st")
                        nc.vector.tensor_mul(st[:, :n], sg[:, :n], pgs[ci][:, :n])
                        nc.vector.tensor_mul(
                            hid[:, im, c0:c0 + n], st[:, :n], pus[ci][:, :n]
                        )

                # ---- phase 2: outT[m, c] = sum_i hidT[i, c] * WdT ----
                for m0 in range(0, HP, interleave_m):
                    ms = list(range(m0, min(m0 + interleave_m, HP)))
                    wdts = []
                    for m in ms:
                        wdt = wdpool.tile([128, IP * 128], BF16, name="wdt",
                                          tag="wdt")
                        nc.sync.dma_start(wdt[:], wd[m])
                        wdts.append(wdt)
                    pds = {(mi, ci): pspool.tile([128, NMAX], F32, name="psd",
                                                 tag="ps")
                           for mi in range(len(ms)) for ci in range(len(chunks))}
                    for k in range(IP):
                        for mi in range(len(ms)):
                            for ci, (c0, n) in enumerate(chunks):
                                nc.tensor.matmul(
                                    pds[mi, ci][:, :n],
                                    wdts[mi][:, k * 128:(k + 1) * 128],
                                    hid[:, k, c0:c0 + n],
                                    start=(k == 0),
                                    stop=(k == IP - 1),
                                )
                    for mi, m in enumerate(ms):
                        for ci, (c0, n) in enumerate(chunks):
                            ot = otpool.tile([128, NMAX], F32, name="ot",
                                             tag="ot")
                            nc.vector.tensor_copy(ot[:, :n], pds[mi, ci][:, :n])
                            nc.sync.dma_start(out[m, :, c0:c0 + n], ot[:, :n])

            if niter == 1:
                body()
            else:
                with tc.For_i(0, niter, 1) as iv:
                    body(iv)

    nc.compile()
    return nc


def route_and_pack(x, expert_indices, expert_weights, gate_proj, up_proj,
                   down_proj, pad_to=8):
    """Host-side dispatch: group tokens by expert, pack per-core inputs."""
    x = np.asarray(x)
    b, s, h = x.shape
    n_tok = b * s
    xf = np.ascontiguousarray(x.reshape(n_tok, h), dtype=np.float32)
    idx = np.asarray(expert_indices).reshape(n_tok, -1).astype(np.int64)
    wts = np.asarray(expert_weights).reshape(n_tok, -1).astype(np.float32)

    # combine[n, e] = sum of slot weights of token n routed to expert e
    combine = np.zeros((n_tok, E), np.float32)
    np.add.at(combine, (np.arange(n_tok)[:, None], idx), wts)

    toks = [np.nonzero(combine[:, e])[0] for e in range(E)]
    counts = [len(t) for t in toks]
    C = max(counts)
    C = ((C + pad_to - 1) // pad_to) * pad_to

    xf_bf = xf.astype(_bf16)
    in_maps = []
    for e in range(E):
        tok_p = np.zeros(C, dtype=np.int64)
        tok_p[:counts[e]] = toks[e]
        xe = xf_bf[tok_p]                                   # [C, H]
        xp = np.ascontiguousarray(xe.reshape(C, HP, 128).transpose(2, 1, 0))
        ag = np.asarray(gate_proj[e], dtype=np.float32)      # [I, H]
        au = np.asarray(up_proj[e], dtype=np.float32)        # [I, H]
        ad = np.asarray(down_proj[e], dtype=np.float32)      # [H, I]
        wg = np.ascontiguousarray(
            ag.reshape(IP, 128, HP, 128).transpose(0, 3, 2, 1).astype(_bf16)
        ).reshape(IP, 128, HP * 128)
        wu = np.ascontiguousarray(
            au.reshape(IP, 128, HP, 128).transpose(0, 3, 2, 1).astype(_bf16)
        ).reshape(IP, 128, HP * 128)
        wd = np.ascontiguousarray(
            ad.reshape(HP, 128, IP, 128).transpose(0, 3, 2, 1).astype(_bf16)
        ).reshape(HP, 128, IP * 128)
        in_maps.append({"xp": xp, "wg": wg, "wu": wu, "wd": wd})

    return {
        "in_maps": in_maps,
        "toks": toks,
        "counts": counts,
        "combine": combine,
        "C": C,
        "shape": (b, s, h),
    }


def combine_results(per_core_out, rp, out_dtype=np.float32):
    """per_core_out[e]: [HP, 128, C] f32 -> full [B, S, H] output."""
    b, s, h = rp["shape"]
    n_tok = b * s
    outf = np.zeros((n_tok, h), np.float32)
    for e in range(E):
        cnt = rp["counts"][e]
        if cnt == 0:
            continue
        ye = np.asarray(per_core_out[e])                     # [HP, 128, C]
        ye = ye.transpose(2, 0, 1).reshape(-1, h)[:cnt]      # [cnt, H]
        tok = rp["toks"][e]
        outf[tok] += ye * rp["combine"][tok, e][:, None]
    return outf.reshape(b, s, h).astype(out_dtype)


def kernel(x, expert_indices, expert_weights, gate_proj, up_proj, down_proj):
    rp = route_and_pack(x, expert_indices, expert_weights,
                        gate_proj, up_proj, down_proj)
    nc = build_program(rp["C"])
    res = run_bass_kernel_spmd(nc, rp["in_maps"], core_ids=list(range(E)))
    per_core_out = [res.results[e]["out"] for e in range(E)]
    return combine_results(per_core_out, rp, out_dtype=np.asarray(x).dtype)



# revision 20
# speedup vs baseline: 1.0443x; 1.0443x over previous
"""MoE routing kernel for one TRN2 chip (8 NeuronCores).

Strategy: expert parallelism, one expert per core. Host-side dispatch:
tokens are grouped by expert (deduped via per-(token,expert) combine
weights), padded to a common capacity C, and packed into PE-friendly
layouts. Each core runs the full gate/up/silu/mul/down MLP for its
expert in bf16 (f32 PSUM accumulation); the host applies combine
weights and scatter-adds into the final output.

All matmuls keep tokens on the moving (free) dimension:
  gate/up: psum[I-chunk 128, ctok]  = Wg_tile[K=H-chunk, M=I-chunk].T @ xT[K, ctok]
  down:    psum[H-chunk 128, ctok]  = Wd_tile[K=I-chunk, M=H-chunk].T @ hidT[K, ctok]
so no on-chip transposes are needed anywhere.

Perf notes (measured on hw):
- The PE column-issue rate here is ~1.86-1.89 Gcols/s regardless of dtype
  (fp8 in the same pattern is only ~1.6% faster), and this kernel streams
  within ~2% of it; the structure is PE-bound at that rate.
- head=True + xbufs=2 land the first gate/up weight tiles before the bulk
  x DMA and double-buffer x, so the x load of iteration n+1 overlaps the
  tail of n (~1.4% faster steady-state, also trims single-shot head stall).
- Redundant back-to-back InstLdweights (same stationary AP) are pruned via
  a tile_legalize wrapper; measured neutral on hw (loads are pipelined)
  but strictly fewer instructions.
"""

import numpy as np
import ml_dtypes

import concourse.bacc as bacc
import concourse.mybir as mybir
import concourse.tile as tile
from concourse.bass_utils import run_bass_kernel_spmd


def _prune_redundant_ldweights(ordered_by_block):
    """Drop InstLdweights whose stationary AP is identical to the weights
    already resident in the PE array (loaded by the previous Ldweights in the
    same block, with only Matmults in between). Runs on tile_legalize output,
    before semaphore assignment, so sync stays consistent. The PE array
    content can't change between the kept load and the elided one: any SBUF
    overwrite of the weight buffer is ordered after the consuming Matmults,
    which still carry the weights AP in their ins.
    """
    n_pruned = 0
    for bb, insts in list(ordered_by_block.items()):
        loaded = None
        out = []
        for inst in insts:
            tn = type(inst).__name__
            if tn == "InstLdweights":
                sig = (
                    str(inst.ins[0]),
                    str(inst.tile_position),
                    str(inst.tile_size),
                    str(inst.perf_mode),
                    str(inst.is_transpose),
                )
                if sig == loaded:
                    n_pruned += 1
                    continue
                loaded = sig
            elif tn == "InstMatmult":
                if inst.is_transpose:
                    loaded = None  # transpose clobbers the array
            out.append(inst)
        ordered_by_block[bb] = out
    return n_pruned


class _LegalizeWithPrune:
    def __init__(self, orig):
        self.orig = orig

    def __call__(self, ordered, nc):
        out = self.orig(ordered, nc)
        _prune_redundant_ldweights(out)
        return out

H = 1024
I = 4096
E = 8
HP = H // 128   # 8  H-chunks
IP = I // 128   # 32 I-chunks
NMAX = 512      # matmul moving free-dim chunk (one PSUM bank of f32)

BF16 = mybir.dt.bfloat16
F32 = mybir.dt.float32
_bf16 = ml_dtypes.bfloat16

_PROGRAM_CACHE = {}


def _chunks(C):
    out = []
    c0 = 0
    while c0 < C:
        n = min(NMAX, C - c0)
        out.append((c0, n))
        c0 += n
    return out


def build_program(C, niter=1, prune=True, order="gguu", interleave_m=1,
                  balance=False, dtype=None, head=True, xbufs=2, **kw):
    """One-core program (SPMD across 8 cores). C = token capacity per core."""
    key = (C, niter, prune, order, interleave_m, balance, str(dtype),
           head, xbufs, tuple(sorted(kw.items())))
    if key in _PROGRAM_CACHE:
        return _PROGRAM_CACHE[key]

    _orig_legalize = tile.tile_legalize
    if prune:
        tile.tile_legalize = _LegalizeWithPrune(_orig_legalize)
    try:
        nc = _build_program_inner(C, niter, order, interleave_m, balance,
                                  dtype or BF16, head=head, xbufs=xbufs, **kw)
    finally:
        tile.tile_legalize = _orig_legalize
    _PROGRAM_CACHE[key] = nc
    return nc


def _build_program_inner(C, niter, order="gguu", interleave_m=1,
                         balance=False, BF16=BF16, xbufs=1, wbufs=4,
                         dbufs=3, head=False):
    nc = bacc.Bacc("TRN2", target_bir_lowering=False, debug=False, num_devices=8)
    xp = nc.dram_tensor("xp", [128, HP, C], BF16, kind="ExternalInput").ap()
    wg = nc.dram_tensor("wg", [IP, 128, HP * 128], BF16, kind="ExternalInput").ap()
    wu = nc.dram_tensor("wu", [IP, 128, HP * 128], BF16, kind="ExternalInput").ap()
    wd = nc.dram_tensor("wd", [HP, 128, IP * 128], BF16, kind="ExternalInput").ap()
    out = nc.dram_tensor("out", [HP, 128, C], F32, kind="ExternalOutput").ap()

    if balance:
        half = ((C + 15) // 16) * 8
        chunks = [(0, half), (half, C - half)]
    else:
        chunks = _chunks(C)

    with tile.TileContext(nc) as tc:
        with (
            tc.tile_pool(name="xpool", bufs=xbufs) as xpool,
            tc.tile_pool(name="hpool", bufs=1) as hpool,
            tc.tile_pool(name="wgpool", bufs=wbufs) as wgpool,
            tc.tile_pool(name="wupool", bufs=wbufs) as wupool,
            tc.tile_pool(name="wdpool", bufs=dbufs) as wdpool,
            tc.tile_pool(name="sgpool", bufs=3) as sgpool,
            tc.tile_pool(name="stpool", bufs=3) as stpool,
            tc.tile_pool(name="otpool", bufs=3) as otpool,
            tc.tile_pool(name="pspool", bufs=8, space="PSUM") as pspool,
        ):

            def body(_iv=None):
                # Optionally land the first weight tiles before the bulk x
                # DMA so the PE's first Ldweights isn't queued behind 2MB.
                pre_w = []
                if head:
                    wgt0 = wgpool.tile([128, HP * 128], BF16, name="wgt",
                                       tag="wgt")
                    nc.sync.dma_start(wgt0[:], wg[0])
                    wut0 = wupool.tile([128, HP * 128], BF16, name="wut",
                                       tag="wut")
                    nc.sync.dma_start(wut0[:], wu[0])
                    pre_w = [(wgt0, wut0)]
                xs = xpool.tile([128, HP, C], BF16, name="xs", tag="xs")
                for k in range(HP):
                    nc.sync.dma_start(xs[:, k, :], xp[:, k, :])
                hid = hpool.tile([128, IP, C], BF16, name="hid", tag="hid")

                # ---- phase 1: hidT[i, c] = silu(gateT) * upT ----
                # Token chunks innermost so each loaded weight tile serves
                # all chunks before the PE switches weights.
                for im in range(IP):
                    if im < len(pre_w):
                        wgt, wut = pre_w[im]
                    else:
                        wgt = wgpool.tile([128, HP * 128], BF16, name="wgt",
                                          tag="wgt")
                        nc.sync.dma_start(wgt[:], wg[im])
                        wut = wupool.tile([128, HP * 128], BF16, name="wut",
                                          tag="wut")
                        nc.sync.dma_start(wut[:], wu[im])
                    pgs = [pspool.tile([128, NMAX], F32, name="psg", tag="ps")
                           for _ in chunks]
                    pus = [pspool.tile([128, NMAX], F32, name="psu", tag="ps")
                           for _ in chunks]
                    for k in range(HP):
                        if order == "gguu":
                            seq = [(pgs, wgt, ci) for ci in range(len(chunks))]
                            seq += [(pus, wut, ci) for ci in range(len(chunks))]
                        else:  # "gugu": alternate stationary every matmul
                            seq = []
                            for ci in range(len(chunks)):
                                seq.append((pgs, wgt, ci))
                                seq.append((pus, wut, ci))
                        for pss, wt, ci in seq:
                            c0, n = chunks[ci]
                            nc.tensor.matmul(
                                pss[ci][:, :n],
                                wt[:, k * 128:(k + 1) * 128],
                                xs[:, k, c0:c0 + n],
                                start=(k == 0),
                                stop=(k == HP - 1),
                            )
                    for ci, (c0, n) in enumerate(chunks):
                        sg = sgpool.tile([128, NMAX], F32, name="sg", tag="sg")
                        nc.scalar.activation(
                            sg[:, :n], pgs[ci][:, :n],
                            mybir.ActivationFunctionType.Sigmoid,
                        )
                        st = stpool.tile([128, NMAX], F32, name="st", tag="st")
                        nc.vector.tensor_mul(st[:, :n], sg[:, :n], pgs[ci][:, :n])
                        nc.vector.tensor_mul(
                            hid[:, im, c0:c0 + n], st[:, :n], pus[ci][:, :n]
                        )

                # ---- phase 2: outT[m, c] = sum_i hidT[i, c] * WdT ----
                for m0 in range(0, HP, interleave_m):
                    ms = list(range(m0, min(m0 + interleave_m, HP)))
                    wdts = []
                    for m in ms:
                        wdt = wdpool.tile([128, IP * 128], BF16, name="wdt",
                                          tag="wdt")
                        nc.sync.dma_start(wdt[:], wd[m])
                        wdts.append(wdt)
                    pds = {(mi, ci): pspool.tile([128, NMAX], F32, name="psd",
                                                 tag="ps")
                           for mi in range(len(ms)) for ci in range(len(chunks))}
                    for k in range(IP):
                        for mi in range(len(ms)):
                            for ci, (c0, n) in enumerate(chunks):
                                nc.tensor.matmul(
                                    pds[mi, ci][:, :n],
                                    wdts[mi][:, k * 128:(k + 1) * 128],
                                    hid[:, k, c0:c0 + n],
                                    start=(k == 0),
                                    stop=(k == IP - 1),
                                )
                    for mi, m in enumerate(ms):
                        for ci, (c0, n) in enumerate(chunks):
                            ot = otpool.tile([128, NMAX], F32, name="ot",
                                             tag="ot")
                            nc.vector.tensor_copy(ot[:, :n], pds[mi, ci][:, :n])
                            nc.sync.dma_start(out[m, :, c0:c0 + n], ot[:, :n])

            if niter == 1:
                body()
            else:
                with tc.For_i(0, niter, 1) as iv:
                    body(iv)

    nc.compile()
    return nc


def route_and_pack(x, expert_indices, expert_weights, gate_proj, up_proj,
                   down_proj, pad_to=8):
    """Host-side dispatch: group tokens by expert, pack per-core inputs."""
    x = np.asarray(x)
    b, s, h = x.shape
    n_tok = b * s
    xf = np.ascontiguousarray(x.reshape(n_tok, h), dtype=np.float32)
    idx = np.asarray(expert_indices).reshape(n_tok, -1).astype(np.int64)
    wts = np.asarray(expert_weights).reshape(n_tok, -1).astype(np.float32)

    # combine[n, e] = sum of slot weights of token n routed to expert e
    combine = np.zeros((n_tok, E), np.float32)
    np.add.at(combine, (np.arange(n_tok)[:, None], idx), wts)

    toks = [np.nonzero(combine[:, e])[0] for e in range(E)]
    counts = [len(t) for t in toks]
    C = max(counts)
    C = ((C + pad_to - 1) // pad_to) * pad_to

    xf_bf = xf.astype(_bf16)
    in_maps = []
    for e in range(E):
        tok_p = np.zeros(C, dtype=np.int64)
        tok_p[:counts[e]] = toks[e]
        xe = xf_bf[tok_p]                                   # [C, H]
        xp = np.ascontiguousarray(xe.reshape(C, HP, 128).transpose(2, 1, 0))
        ag = np.asarray(gate_proj[e], dtype=np.float32)      # [I, H]
        au = np.asarray(up_proj[e], dtype=np.float32)        # [I, H]
        ad = np.asarray(down_proj[e], dtype=np.float32)      # [H, I]
        wg = np.ascontiguousarray(
            ag.reshape(IP, 128, HP, 128).transpose(0, 3, 2, 1).astype(_bf16)
        ).reshape(IP, 128, HP * 128)
        wu = np.ascontiguousarray(
            au.reshape(IP, 128, HP, 128).transpose(0, 3, 2, 1).astype(_bf16)
        ).reshape(IP, 128, HP * 128)
        wd = np.ascontiguousarray(
            ad.reshape(HP, 128, IP, 128).transpose(0, 3, 2, 1).astype(_bf16)
        ).reshape(HP, 128, IP * 128)
        in_maps.append({"xp": xp, "wg": wg, "wu": wu, "wd": wd})

    return {
        "in_maps": in_maps,
        "toks": toks,
        "counts": counts,
        "combine": combine,
        "C": C,
        "shape": (b, s, h),
    }


def combine_results(per_core_out, rp, out_dtype=np.float32):
    """per_core_out[e]: [HP, 128, C] f32 -> full [B, S, H] output."""
    b, s, h = rp["shape"]
    n_tok = b * s
    outf = np.zeros((n_tok, h), np.float32)
    for e in range(E):
        cnt = rp["counts"][e]
        if cnt == 0:
            continue
        ye = np.asarray(per_core_out[e])                     # [HP, 128, C]
        ye = ye.transpose(2, 0, 1).reshape(-1, h)[:cnt]      # [cnt, H]
        tok = rp["toks"][e]
        outf[tok] += ye * rp["combine"][tok, e][:, None]
    return outf.reshape(b, s, h).astype(out_dtype)


def kernel(x, expert_indices, expert_weights, gate_proj, up_proj, down_proj):
    rp = route_and_pack(x, expert_indices, expert_weights,
                        gate_proj, up_proj, down_proj)
    nc = build_program(rp["C"])
    res = run_bass_kernel_spmd(nc, rp["in_maps"], core_ids=list(range(E)))
    per_core_out = [res.results[e]["out"] for e in range(E)]
    return combine_results(per_core_out, rp, out_dtype=np.asarray(x).dtype)

